# revision 4
# baseline (speedup 1.0000x reference)
"""Trainium2 Bass kernel for block-local causal multi-head attention.

Problem (hardcoded): x [4, 4096, 1024] f32, 4x [1024,1024] projection
weights + biases. Sequence is split into independent causal blocks of 256.
B*nb = 64 blocks -> 8 blocks per core across 8 NeuronCores (data parallel,
weights replicated, no collectives).

Per-core device layout is fully "transposed" (feature-major) so that no
input transposes are needed on device:
  - host ships xT = x_shard.T  [1024, 2048] bf16
  - Q^T, K^T = Wq/Wk (stationary) .T @ xT           -> [1024, 2048]
  - V (row-major) = xT (stationary) .T @ Wv          -> [2048, 1024]
  - S = q^T(stationary).T @ k^T  per (block, head)   -> natural [sq, sk]
  - softmax along free dim (exp on ACT with fused 1/8 scale + accum rowsum,
    reciprocal + per-partition normalize on DVE)
  - P^T via PE transpose (bf16), PV: lhsT=V slice, rhs=P^T -> attn^T
  - y^T = Wo (stationary) .T @ attn^T                -> [1024, 2048] f32
  - host transposes y^T back.
Biases: bq/bk/bo applied as fused per-partition ACT bias on PSUM->SBUF
evacuation; bv is folded into bo on host (softmax rows sum to 1, so
attn(v + bv) = attn(v) + bv and bv @ Wo is a constant added to bo).
"""

import sys

if "/opt/trn_rl_repo" not in sys.path:
    sys.path.insert(0, "/opt/trn_rl_repo")

import ml_dtypes
import numpy as np

import concourse.bass as bass
import concourse.mybir as mybir
import concourse.tile as tile

N_CORES = 8
D = 1024
BLK = 256
NH = 16
DH = 64
B, S = 4, 4096
N_BLOCKS = B * (S // BLK)  # 64
BLOCKS_PER_CORE = N_BLOCKS // N_CORES  # 8
SEQ = BLOCKS_PER_CORE * BLK  # 2048 seq positions per core
N_PAIRS = BLOCKS_PER_CORE // 2  # process blocks in pairs of 512 seq cols
MASK_NEG = -1.0e4  # pre-scale additive mask; exp(0.125 * -1e4) == 0.0

BF16 = ml_dtypes.bfloat16
AF = mybir.ActivationFunctionType
dt = mybir.dt

_cache = {}


def _legalize_waits(nc, max_waits=1):
    """This environment's walrus build rejects instructions with more than
    one sync-wait command ("Too many sync wait commands"). Split extra waits
    onto same-engine NoOps inserted immediately before the instruction —
    semantically identical (engine streams are in-order)."""
    fn = nc.m.functions[0]
    k = 0
    for blk in fn.blocks:
        insts = blk.instructions
        if not any(
            i.sync_info is not None and len(i.sync_info.on_wait) > max_waits
            for i in insts
        ):
            continue
        new = []
        for inst in insts:
            si = inst.sync_info
            if si is not None and len(si.on_wait) > max_waits:
                waits = list(si.on_wait)
                for w in waits[:-max_waits]:
                    k += 1
                    new.append(
                        mybir.InstNoOp(
                            name=f"I-wsplit-{k}",
                            engine=inst.engine,
                            sync_info=mybir.SyncInfo(on_wait=[w], on_update=[]),
                        )
                    )
                inst.sync_info = mybir.SyncInfo(
                    on_wait=waits[-max_waits:], on_update=list(si.on_update)
                )
            new.append(inst)
        blk.instructions = new


def _build_nc():
    nc = bass.Bass(
        "TRN2", target_bir_lowering=True, debug=False, enable_asserts=False
    )

    xT = nc.dram_tensor("xT", [D, SEQ], dt.bfloat16, kind="ExternalInput").ap()
    wq = nc.dram_tensor("wq", [D, D], dt.bfloat16, kind="ExternalInput").ap()
    wk = nc.dram_tensor("wk", [D, D], dt.bfloat16, kind="ExternalInput").ap()
    wv = nc.dram_tensor("wv", [D, D], dt.bfloat16, kind="ExternalInput").ap()
    wo = nc.dram_tensor("wo", [D, D], dt.bfloat16, kind="ExternalInput").ap()
    bqt = nc.dram_tensor("bqt", [128, 8], dt.float32, kind="ExternalInput").ap()
    bkt = nc.dram_tensor("bkt", [128, 8], dt.float32, kind="ExternalInput").ap()
    bot = nc.dram_tensor("bot", [128, 8], dt.float32, kind="ExternalInput").ap()
    mng = nc.dram_tensor("mng", [128, 128], dt.float32, kind="ExternalInput").ap()
    idn = nc.dram_tensor("idn", [128, 128], dt.bfloat16, kind="ExternalInput").ap()
    yT = nc.dram_tensor("yT", [D, SEQ], dt.float32, kind="ExternalOutput").ap()

    with tile.TileContext(nc) as tc:
        with (
            tc.tile_pool(name="const", bufs=1) as constp,
            tc.tile_pool(name="xw", bufs=1) as xwp,
            tc.tile_pool(name="qkv", bufs=2) as qkvp,
            tc.tile_pool(name="attn", bufs=3) as attnp,
            tc.tile_pool(name="atp", bufs=2) as atp,
            tc.tile_pool(name="yp", bufs=4) as yp,
            tc.tile_pool(name="ps_proj", bufs=2, space="PSUM") as pproj,
            tc.tile_pool(name="ps_s", bufs=2, space="PSUM") as ps_s,
            tc.tile_pool(name="ps_t", bufs=2, space="PSUM") as ps_t,
            tc.tile_pool(name="ps_o", bufs=2, space="PSUM") as ps_o,
        ):
            ident = constp.tile([128, 128], dt.bfloat16, name="ident")
            nc.sync.dma_start(out=ident[:], in_=idn)
            mneg = constp.tile([128, 128], dt.float32, name="mneg")
            nc.sync.dma_start(out=mneg[:], in_=mng)
            bq_sb = constp.tile([128, 8], dt.float32, name="bq_sb")
            nc.sync.dma_start(out=bq_sb[:], in_=bqt)
            bk_sb = constp.tile([128, 8], dt.float32, name="bk_sb")
            nc.sync.dma_start(out=bk_sb[:], in_=bkt)
            bo_sb = constp.tile([128, 8], dt.float32, name="bo_sb")
            nc.sync.dma_start(out=bo_sb[:], in_=bot)

            xts = []
            for k in range(8):
                t = xwp.tile([128, SEQ], dt.bfloat16, name=f"xt{k}", tag=f"xt{k}")
                nc.sync.dma_start(out=t[:], in_=xT[k * 128 : (k + 1) * 128, :])
                xts.append(t)

            def load_w(wap, nm):
                ts_ = []
                for k in range(8):
                    t = xwp.tile(
                        [128, D], dt.bfloat16, name=f"{nm}{k}", tag=f"{nm}{k}"
                    )
                    nc.sync.dma_start(out=t[:], in_=wap[k * 128 : (k + 1) * 128, :])
                    ts_.append(t)
                return ts_

            wqs = load_w(wq, "wq")
            wks = load_w(wk, "wk")
            wvs = load_w(wv, "wv")
            wos = load_w(wo, "wo")

            for p in range(N_PAIRS):
                pc0 = p * 512

                # --- QKV projections for this pair of blocks (512 seq cols)
                qts, kts = [], []
                for wlist, b_sb, outlist, tg in (
                    (wqs, bq_sb, qts, "q"),
                    (wks, bk_sb, kts, "k"),
                ):
                    for m in range(8):
                        ps = pproj.tile(
                            [128, 512], dt.float32, name=f"ps_{tg}{m}", tag="proj"
                        )
                        for k in range(8):
                            nc.tensor.matmul(
                                ps[:],
                                wlist[k][:, m * 128 : (m + 1) * 128],
                                xts[k][:, pc0 : pc0 + 512],
                                start=(k == 0),
                                stop=(k == 7),
                            )
                        sb = qkvp.tile(
                            [128, 512], dt.bfloat16, name=f"{tg}t{m}", tag=f"{tg}{m}"
                        )
                        nc.scalar.activation(
                            sb[:], ps[:], AF.Identity, bias=b_sb[:, m : m + 1]
                        )
                        outlist.append(sb)

                vts = []
                for st in range(4):
                    vt = qkvp.tile(
                        [128, D], dt.bfloat16, name=f"vt{st}", tag=f"v{st}"
                    )
                    for ch in range(2):
                        ps = pproj.tile(
                            [128, 512], dt.float32, name=f"ps_v{st}{ch}", tag="proj"
                        )
                        for k in range(8):
                            nc.tensor.matmul(
                                ps[:],
                                xts[k][:, pc0 + st * 128 : pc0 + (st + 1) * 128],
                                wvs[k][:, ch * 512 : (ch + 1) * 512],
                                start=(k == 0),
                                stop=(k == 7),
                            )
                        nc.vector.tensor_copy(vt[:, ch * 512 : (ch + 1) * 512], ps[:])
                    vts.append(vt)

                # --- attention + output projection per block
                for beta in range(2):
                    bc0 = beta * 256
                    gblk = 2 * p + beta
                    v0 = vts[2 * beta]
                    v1 = vts[2 * beta + 1]

                    ats = [
                        atp.tile(
                            [128, 256], dt.bfloat16, name=f"at{kk}", tag=f"at{kk}"
                        )
                        for kk in range(8)
                    ]

                    for h in range(NH):
                        ht, hp = h // 2, (h % 2) * 64
                        hs = slice(h * 64, (h + 1) * 64)

                        # scores, natural [sq, sk]; cols 0:128 = sq0 x sk0,
                        # cols 128:384 = sq1 x sk0:2
                        S_ = ps_s.tile([128, 384], dt.float32, name="S_", tag="s")
                        nc.tensor.matmul(
                            S_[:, 0:128],
                            qts[ht][hp : hp + 64, bc0 : bc0 + 128],
                            kts[ht][hp : hp + 64, bc0 : bc0 + 128],
                            start=True,
                            stop=True,
                        )
                        nc.tensor.matmul(
                            S_[:, 128:384],
                            qts[ht][hp : hp + 64, bc0 + 128 : bc0 + 256],
                            kts[ht][hp : hp + 64, bc0 : bc0 + 256],
                            start=True,
                            stop=True,
                        )
                        # causal mask on the diagonal chunks
                        nc.vector.tensor_tensor(
                            S_[:, 0:128], S_[:, 0:128], mneg[:], mybir.AluOpType.add
                        )
                        nc.vector.tensor_tensor(
                            S_[:, 256:384],
                            S_[:, 256:384],
                            mneg[:],
                            mybir.AluOpType.add,
                        )
                        # exp(S/8) with fused rowsum
                        E_ = attnp.tile([128, 384], dt.bfloat16, name="E_", tag="e")
                        dd = attnp.tile([128, 2], dt.float32, name="dd", tag="d")
                        nc.scalar.activation(
                            E_[:, 0:128],
                            S_[:, 0:128],
                            AF.Exp,
                            scale=0.125,
                            accum_out=dd[:, 0:1],
                        )
                        nc.scalar.activation(
                            E_[:, 128:384],
                            S_[:, 128:384],
                            AF.Exp,
                            scale=0.125,
                            accum_out=dd[:, 1:2],
                        )
                        rr = attnp.tile([128, 2], dt.float32, name="rr", tag="r")
                        nc.vector.reciprocal(rr[:], dd[:])
                        P_ = attnp.tile([128, 384], dt.bfloat16, name="P_", tag="p")
                        nc.vector.tensor_scalar_mul(
                            P_[:, 0:128], E_[:, 0:128], rr[:, 0:1]
                        )
                        nc.vector.tensor_scalar_mul(
                            P_[:, 128:384], E_[:, 128:384], rr[:, 1:2]
                        )
                        # transpose P -> P^T (PE, bf16 PSUM)
                        T_ = ps_t.tile([128, 384], dt.bfloat16, name="T_", tag="t")
                        nc.tensor.transpose(T_[:, 0:128], P_[:, 0:128], ident[:])
                        nc.tensor.transpose(T_[:, 128:256], P_[:, 128:256], ident[:])
                        nc.tensor.transpose(T_[:, 256:384], P_[:, 256:384], ident[:])
                        PT0 = attnp.tile(
                            [128, 256], dt.bfloat16, name="PT0", tag="pt0"
                        )
                        PT1 = attnp.tile(
                            [128, 128], dt.bfloat16, name="PT1", tag="pt1"
                        )
                        nc.vector.tensor_copy(PT0[:], T_[:, 0:256])
                        nc.scalar.copy(PT1[:], T_[:, 256:384])
                        # PV: attn^T head slice [64, 256]
                        O_ = ps_o.tile([64, 256], dt.float32, name="O_", tag="o")
                        nc.tensor.matmul(
                            O_[:, 0:128], v0[:, hs], PT0[:, 0:128], start=True, stop=True
                        )
                        nc.tensor.matmul(
                            O_[:, 128:256],
                            v0[:, hs],
                            PT0[:, 128:256],
                            start=True,
                            stop=False,
                        )
                        nc.tensor.matmul(
                            O_[:, 128:256], v1[:, hs], PT1[:], start=False, stop=True
                        )
                        nc.scalar.copy(ats[ht][hp : hp + 64, :], O_[:])

                    # output projection y^T = Wo.T @ attn^T for this block
                    for m in range(8):
                        ps = pproj.tile(
                            [128, 256], dt.float32, name=f"ps_y{m}", tag="proj"
                        )
                        for k in range(8):
                            nc.tensor.matmul(
                                ps[:],
                                wos[k][:, m * 128 : (m + 1) * 128],
                                ats[k][:],
                                start=(k == 0),
                                stop=(k == 7),
                            )
                        yt = yp.tile([128, 256], dt.float32, name=f"yt{m}", tag="yt")
                        nc.scalar.activation(
                            yt[:], ps[:], AF.Identity, bias=bo_sb[:, m : m + 1]
                        )
                        nc.sync.dma_start(
                            out=yT[
                                m * 128 : (m + 1) * 128,
                                gblk * 256 : (gblk + 1) * 256,
                            ],
                            in_=yt[:],
                        )
    _legalize_waits(nc)
    return nc


def get_nc():
    if "nc" not in _cache:
        _cache["nc"] = _build_nc()
    return _cache["nc"]


def make_in_maps(x, Wq, bq, Wk, bk, Wv, bv, Wo, bo):
    """Host-side sharding/packing. Returns list of 8 per-core input dicts."""
    x = np.asarray(x, np.float32)
    Wq, Wk, Wv, Wo = (np.asarray(w, np.float32) for w in (Wq, Wk, Wv, Wo))
    bq, bk, bv, bo = (np.asarray(b, np.float32) for b in (bq, bk, bv, bo))

    # softmax rows sum to 1 -> attn @ (V + bv) = attn @ V + bv; fold into bo
    bo2 = bo + bv @ Wo

    wq_b, wk_b, wv_b, wo_b = (w.astype(BF16) for w in (Wq, Wk, Wv, Wo))
    bqt = np.ascontiguousarray(bq.reshape(8, 128).T)
    bkt = np.ascontiguousarray(bk.reshape(8, 128).T)
    bot = np.ascontiguousarray(bo2.reshape(8, 128).T)
    tri = np.tril(np.ones((128, 128), np.float32))
    mng = np.where(tri > 0, np.float32(0.0), np.float32(MASK_NEG))
    idn = np.eye(128, dtype=BF16)

    xb = x.reshape(N_BLOCKS, BLK, D)
    in_maps = []
    for c in range(N_CORES):
        xc = xb[c * BLOCKS_PER_CORE : (c + 1) * BLOCKS_PER_CORE].reshape(SEQ, D)
        xTc = np.ascontiguousarray(xc.T.astype(BF16))
        in_maps.append(
            {
                "xT": xTc,
                "wq": wq_b,
                "wk": wk_b,
                "wv": wv_b,
                "wo": wo_b,
                "bqt": bqt,
                "bkt": bkt,
                "bot": bot,
                "mng": mng,
                "idn": idn,
            }
        )
    return in_maps


def assemble_output(yT_list):
    """yT_list: per-core [1024, 2048] f32 -> full [4, 4096, 1024] f32."""
    y = np.empty((N_BLOCKS, BLK, D), np.float32)
    for c in range(N_CORES):
        y[c * BLOCKS_PER_CORE : (c + 1) * BLOCKS_PER_CORE] = (
            yT_list[c].T.reshape(BLOCKS_PER_CORE, BLK, D)
        )
    return np.ascontiguousarray(y.reshape(B, S, D))


def kernel(x, Wq, bq, Wk, bk, Wv, bv, Wo, bo):
    from concourse.bass_utils import run_bass_kernel_spmd

    in_maps = make_in_maps(x, Wq, bq, Wk, bk, Wv, bv, Wo, bo)
    nc = get_nc()
    res = run_bass_kernel_spmd(nc, in_maps, list(range(N_CORES)))
    return assemble_output([res.results[c]["yT"] for c in range(N_CORES)])
